# revision 1
# baseline (speedup 1.0000x reference)
"""Trainium2 Bass kernel for nn_BasisFunction2D (2-D basis-function embedding lookup).

Reformulation: the data-dependent bilinear interpolation over a 16x16 grid of
per-(ix,iz) tables is expressed as dense "hat-function" interpolation matrices

    V[(q,iz), b] = hat_q(z[iz,b])      (z-side weights, 2 nonzeros per column)
    U[b, (p,ix)] = hat_p(x[ix,b])      (x-side weights, transposed layout)

so that   out[o,b] = sum_{k,m} V[k,b] * G_o[k,m] * U[b,m]
with      G_o[(q,iz),(p,ix)] = func_parameter[p,q,o,ix,iz].

Per core (output dim o sharded 8-ways, 8 o's per core):
  stage 1 (TensorE, fp32r):  C_o[b, m] = sum_k V[k,b] G_o[k,m]   (PSUM, K tiled by 128)
  stage 2 (VectorE):         out[b,o] = sum_m C_o[b,m] * U[b,m]  (fused mult+reduce)

The hat functions (including the reference's linear tail extrapolation) are built
on-device from affine+relu+min ops; host only reshapes/replicates inputs.
"""

import numpy as np

import concourse.bass as bass
import concourse.bacc as bacc_mod
import concourse.tile as tile
from concourse import mybir
from concourse.bass_utils import run_bass_kernel_spmd

F32 = mybir.dt.float32
F32R = mybir.dt.float32r
BF16 = mybir.dt.bfloat16
AF = mybir.ActivationFunctionType
ALU = mybir.AluOpType

NCORES = 8
NG = 16            # grid bins
NQ = 17            # grid corners per axis
IX = 32
IZ = 32
OUT = 64
B = 512
OSH = OUT // NCORES          # outputs per core = 8
K = NQ * IZ                  # 544 contraction rows (q,iz)
M = NQ * IX                  # 544 free cols (p,ix)
NSPL = 272                   # N split halves (both >=256 for fp32r full rate)
BIG = 1e30
KCH = [(0, 128), (128, 128), (256, 128), (384, 128), (512, 32)]
NBC = B // 128               # 4 batch chunks

_NC_CACHE = {}


def _build_nc(bd, il, debug=False):
    """Build the single-core Bass/Tile program (identical across cores)."""
    bd = [float(v) for v in bd]
    il = [float(v) for v in il]

    nc = bacc_mod.Bacc(None, target_bir_lowering=False)
    gt_d = nc.dram_tensor("gt", [OSH, K, M], BF16, kind="ExternalInput")
    zv_d = nc.dram_tensor("zv", [K, B + 8], F32, kind="ExternalInput")
    zrpad_d = nc.dram_tensor("zrpad", [B, M + 32], F32, kind="ExternalInput")
    out_d = nc.dram_tensor("out", [B, OSH], F32, kind="ExternalOutput")
    if debug:
        dbgv_d = nc.dram_tensor("dbg_v", [K, B], F32, kind="ExternalOutput")
        dbgu_d = nc.dram_tensor("dbg_u", [B, M], F32, kind="ExternalOutput")
        dbgc_d = nc.dram_tensor("dbg_c", [B, M], F32, kind="ExternalOutput")

    with tile.TileContext(nc) as tc:
        with (
            tc.tile_pool(name="per", bufs=1) as per,       # persistent tiles
            tc.tile_pool(name="tmp", bufs=1) as tmp,       # V/U build scratch
            tc.tile_pool(name="sc", bufs=6) as sc,         # stt mandatory outs
            tc.tile_pool(name="ac", bufs=6) as ac,         # [128,1] accumulators
            tc.tile_pool(name="ps", bufs=8, space="PSUM") as ps,
        ):
            # PE warmup: dependency-free dummy matmuls overlap the DMA phase and
            # flip the HAM clock-gate to 8/8 before the real matmuls start.
            wt = per.tile([128, B], BF16, tag="warm", name="wt")
            nc.vector.memset(wt[:], 0.0)
            wps = ps.tile([128, B], F32, tag="ps", name="wps")
            for _ in range(10):
                nc.tensor.matmul(wps[:], wt[:, 0:128], wt[:], start=True, stop=True)

            # ---------------- input loads ----------------
            zv_sb, V_sb = [], []
            for kci, (r0, rows) in enumerate(KCH):
                zt = per.tile([rows, B + 8], F32, tag=f"zv{kci}", name=f"zv{kci}")
                nc.sync.dma_start(zt[:], zv_d[r0:r0 + rows, :])
                zv_sb.append(zt)
                V_sb.append(per.tile([rows, B], BF16, tag=f"V{kci}", name=f"V{kci}"))
            zrep_sb = [t[:, 0:B] for t in zv_sb]
            vcoef_sb = [t[:, B:B + 8] for t in zv_sb]

            zrpad_sb, U_sb, outT_sb = [], [], []
            for bc in range(NBC):
                pt = per.tile([128, M + 32], F32, tag=f"zrpad{bc}", name=f"zrpad{bc}")
                nc.sync.dma_start(pt[:], zrpad_d[bc * 128:(bc + 1) * 128, :])
                zrpad_sb.append(pt)
                U_sb.append(per.tile([128, M], F32, tag=f"U{bc}", name=f"U{bc}"))
                outT_sb.append(per.tile([128, OSH], F32, tag=f"outT{bc}", name=f"outT{bc}"))

            G_sb = []
            for o in range(OSH):
                row = []
                for kci, (r0, rows) in enumerate(KCH):
                    gtile = per.tile([rows, M], BF16, tag=f"G{o}_{kci}", name=f"G{o}_{kci}")
                    nc.sync.dma_start(gtile[:], gt_d[o, r0:r0 + rows, :])
                    row.append(gtile)
                G_sb.append(row)

            # ---------------- build V[(q,iz), b] ----------------
            # rows r = q*32 + iz;  L = (z - bd[q-1])*il[q-1], R = (bd[q+1] - z)*il[q]
            # V = min(relu(L), relu(R)); per-row (scale,bias) from vcoef cols.
            for kci, (r0, rows) in enumerate(KCH):
                lp = tmp.tile([rows, B], F32, tag=f"vL{kci}", name=f"vL{kci}")
                nc.scalar.activation(lp[:], zrep_sb[kci], AF.Relu,
                                     bias=vcoef_sb[kci][:, 1:2], scale=vcoef_sb[kci][:, 0:1])
                rp = tmp.tile([rows, B], F32, tag=f"vR{kci}", name=f"vR{kci}")
                nc.scalar.activation(rp[:], zrep_sb[kci], AF.Relu,
                                     bias=vcoef_sb[kci][:, 3:4], scale=vcoef_sb[kci][:, 2:3])
                nc.vector.tensor_tensor(V_sb[kci][:], lp[:], rp[:], ALU.min)

            # tail fixes: q=1 keeps L un-relu'd (left extrapolation), q=15 keeps R raw.
            fl = tmp.tile([32, B], F32, tag="vfq1L", name="vfixq1L")
            nc.scalar.activation(fl[:], zv_sb[0][32:64, 0:B], AF.Copy,
                                 bias=float(-bd[0] * il[0]), scale=float(il[0]))
            fr = tmp.tile([32, B], F32, tag="vfq1R", name="vfixq1R")
            nc.scalar.activation(fr[:], zv_sb[0][32:64, 0:B], AF.Relu,
                                 bias=zv_sb[0][32:64, B + 4:B + 5], scale=float(-il[1]))
            nc.vector.tensor_tensor(V_sb[0][32:64, :], fl[:], fr[:], ALU.min)

            fl2 = tmp.tile([32, B], F32, tag="vfq15L", name="vfixq15L")
            nc.scalar.activation(fl2[:], zv_sb[3][96:128, 0:B], AF.Relu,
                                 bias=zv_sb[3][96:128, B + 4:B + 5], scale=float(il[14]))
            fr2 = tmp.tile([32, B], F32, tag="vfq15R", name="vfixq15R")
            nc.scalar.activation(fr2[:], zv_sb[3][96:128, 0:B], AF.Copy,
                                 bias=float(bd[16] * il[15]), scale=float(-il[15]))
            nc.vector.tensor_tensor(V_sb[3][96:128, :], fl2[:], fr2[:], ALU.min)

            # ---------------- build U[b, (p,ix)] (transposed layout) ----------------
            # zrpad col 32+p*32+ix = R_p(x[ix,b]); identity L_p = 1 - R_{p-1} gives the
            # L operand as a 32-col-shifted read of the same tensor.
            for bc in range(NBC):
                lp = tmp.tile([128, M], F32, tag=f"uL{bc}", name=f"uL{bc}")
                nc.scalar.activation(lp[:], zrpad_sb[bc][:, 0:M], AF.Relu,
                                     bias=1.0, scale=-1.0)
                rp = tmp.tile([128, M], F32, tag=f"uR{bc}", name=f"uR{bc}")
                nc.scalar.activation(rp[:], zrpad_sb[bc][:, 32:M + 32], AF.Relu)
                nc.vector.tensor_tensor(U_sb[bc][:], lp[:], rp[:], ALU.min)
                # p=1 fix: L un-relu'd
                fx = tmp.tile([128, 32], F32, tag=f"ufix{bc}", name=f"ufix{bc}")
                nc.scalar.activation(fx[:], zrpad_sb[bc][:, 32:64], AF.Copy,
                                     bias=1.0, scale=-1.0)
                nc.vector.tensor_tensor(U_sb[bc][:, 32:64], fx[:], rp[:, 32:64], ALU.min)
                # p=15 fix: R raw (direct from input)
                nc.vector.tensor_tensor(U_sb[bc][:, 480:512], lp[:, 480:512],
                                        zrpad_sb[bc][:, 512:544], ALU.min)

            if debug:
                for kci, (r0, rows) in enumerate(KCH):
                    nc.sync.dma_start(dbgv_d[r0:r0 + rows, :], V_sb[kci][:])
                for bc in range(NBC):
                    nc.sync.dma_start(dbgu_d[bc * 128:(bc + 1) * 128, :], U_sb[bc][:])

            # ---------------- main loop: matmuls + fused reduce ----------------
            for o in range(OSH):
                for bc in range(NBC):
                    psA = ps.tile([128, NSPL], F32, tag="ps", name=f"psA{o}_{bc}")
                    psB = ps.tile([128, NSPL], F32, tag="ps", name=f"psB{o}_{bc}")
                    for kci, (r0, rows) in enumerate(KCH):
                        lhs = V_sb[kci][0:rows, bc * 128:(bc + 1) * 128]
                        st, sp = (kci == 0), (kci == len(KCH) - 1)
                        nc.tensor.matmul(psA[:], lhs,
                                         G_sb[o][kci][0:rows, 0:NSPL],
                                         start=st, stop=sp)
                        nc.tensor.matmul(psB[:], lhs,
                                         G_sb[o][kci][0:rows, NSPL:M],
                                         start=st, stop=sp)
                    if debug and o == 0:
                        dtmpA = sc.tile([128, NSPL], F32, tag="dbg", name=f"dbgA{bc}")
                        nc.vector.tensor_copy(dtmpA[:], psA[:])
                        nc.sync.dma_start(dbgc_d[bc * 128:(bc + 1) * 128, 0:NSPL], dtmpA[:])
                        dtmpB = sc.tile([128, NSPL], F32, tag="dbg", name=f"dbgB{bc}")
                        nc.vector.tensor_copy(dtmpB[:], psB[:])
                        nc.sync.dma_start(dbgc_d[bc * 128:(bc + 1) * 128, NSPL:M], dtmpB[:])
                    scA = sc.tile([128, NSPL], F32, tag="sc", name=f"scA{o}_{bc}")
                    acc1 = ac.tile([128, 1], F32, tag="ac", name=f"acc1_{o}_{bc}")
                    nc.vector.scalar_tensor_tensor(
                        out=scA[:], in0=psA[:], scalar=1.0, in1=U_sb[bc][:, 0:NSPL],
                        op0=ALU.mult, op1=ALU.mult, accum_out=acc1[:])
                    scB = sc.tile([128, NSPL], F32, tag="sc", name=f"scB{o}_{bc}")
                    acc2 = ac.tile([128, 1], F32, tag="ac2", name=f"acc2_{o}_{bc}")
                    nc.vector.scalar_tensor_tensor(
                        out=scB[:], in0=psB[:], scalar=1.0, in1=U_sb[bc][:, NSPL:M],
                        op0=ALU.mult, op1=ALU.mult, accum_out=acc2[:])
                    nc.vector.tensor_add(outT_sb[bc][:, o:o + 1], acc1[:], acc2[:])

            # ---------------- store ----------------
            for bc in range(NBC):
                nc.sync.dma_start(out_d[bc * 128:(bc + 1) * 128, :], outT_sb[bc][:])

    nc.finalize()
    return nc


def _host_prep(x, z, func_parameter, borders, il):
    x = np.asarray(x, np.float32)
    z = np.asarray(z, np.float32)
    F = np.asarray(func_parameter, np.float32)
    bd = np.asarray(borders, np.float32)
    il = np.asarray(il, np.float32)

    # G_all[o, q*32+iz, p*32+ix] = F[p,q,o,ix,iz]
    import ml_dtypes
    G_all = np.ascontiguousarray(
        F.transpose(2, 1, 4, 0, 3)).reshape(OUT, K, M).astype(ml_dtypes.bfloat16)

    zrep = np.tile(z, (NQ, 1))                                # [544, 512]

    q = np.arange(NQ)
    aL = np.where(q >= 1, il[np.clip(q - 1, 0, NG - 1)], 0.0)
    bL = np.where(q >= 1, -bd[np.clip(q - 1, 0, NQ - 1)] * il[np.clip(q - 1, 0, NG - 1)], BIG)
    aR = np.where(q <= NG - 1, -il[np.clip(q, 0, NG - 1)], 0.0)
    bR = np.where(q <= NG - 1, bd[np.clip(q + 1, 0, NQ - 1)] * il[np.clip(q, 0, NG - 1)], BIG)
    fixb = np.zeros(NQ)
    fixb[1] = bd[2] * il[1]          # V q=1 fix: R-side relu bias
    fixb[15] = -bd[14] * il[14]      # V q=15 fix: L-side relu bias
    zero = np.zeros(NQ)
    vcoef = np.ascontiguousarray(
        np.stack([np.repeat(c.astype(np.float32), IZ)
                  for c in (aL, bL, aR, bR, fixb, zero, zero, zero)], axis=1))

    Rx = np.empty((NQ, IX, B), np.float32)
    for p in range(NG):
        Rx[p] = (bd[p + 1] - x) * il[p]
    Rx[NG] = BIG
    ZR_T = Rx.transpose(2, 0, 1).reshape(B, M)
    zrpad = np.ascontiguousarray(
        np.concatenate([np.full((B, 32), -BIG, np.float32), ZR_T], axis=1))

    zv = np.ascontiguousarray(np.concatenate([zrep, vcoef], axis=1))  # [544, 520]
    return G_all, zv, zrpad, bd, il


def kernel(x, z, func_parameter, borders, inverse_chunk_lengths, _trace=False):
    G_all, zv, zrpad, bd, il = _host_prep(
        x, z, func_parameter, borders, inverse_chunk_lengths)

    key = (bd.tobytes(), il.tobytes())
    if key not in _NC_CACHE:
        _NC_CACHE[key] = _build_nc(bd, il)
    nc = _NC_CACHE[key]

    in_maps = []
    for c in range(NCORES):
        in_maps.append({
            "gt": np.ascontiguousarray(G_all[c * OSH:(c + 1) * OSH]),
            "zv": zv,
            "zrpad": zrpad,
        })

    res = run_bass_kernel_spmd(nc, in_maps, core_ids=list(range(NCORES)),
                               trace=_trace)
    out = np.concatenate([res.results[c]["out"].T for c in range(NCORES)], axis=0)
    out = np.ascontiguousarray(out.astype(np.float32))
    if _trace:
        return out, res
    return out



# revision 8
# speedup vs baseline: 1.0348x; 1.0348x over previous
"""Trainium2 Bass kernel for nn_BasisFunction2D (2-D basis-function embedding lookup).

Reformulation (v2): data-dependent bilinear interpolation over a 16x16 grid of
per-(ix,iz) tables expressed as dense hat-function interpolation matrices

    V[(q,iz), b] = hat_q(z[iz,b])      (z-side weights, 2 nonzeros per column)
    U[(p,ix), b] = hat_p(x[ix,b])      (x-side weights)

with the x-side partition-of-unity fold (sum_p hat_p = 1, exact even in the
linear-extrapolation tails):

    out[o,b] = sum_{m in 512} C_o[b,m] * U[m,b]  +  sum_k V[k,b] * gr_o[k]
    C_o[b,m] = sum_{k in 544} V[k,b] * Ghat_o[k,m]        (PE, bf16, N=512)
    Ghat_o[k,(p,ix)] = G_o[k,(p,ix)] - G_o[k,(16,ix)],  p <= 15
    gr_o[k] = sum_ix G_o[k,(16,ix)]

M=512 makes each (o,bc) accumulation chain exactly one PSUM bank, so chains
pipeline 6 deep and the PE runs gap-free (p-state ramps to 2.4 GHz).  The
side term sum_k V*gr is a tiny padded [32,512] PE chain; its transpose-add
into the output happens on the host (returned as a second output).

Stage 2 alternates per chain between (a) ACT copy PSUM->SBUF bf16 + DVE 2x
fused multiply-reduce and (b) DVE direct fp32 multiply-reduce on PSUM,
balancing ACT and DVE under the PE.
"""

import numpy as np

import concourse.bass as bass
import concourse.bacc as bacc_mod
import concourse.tile as tile
from concourse import mybir
from concourse.bass_utils import run_bass_kernel_spmd

F32 = mybir.dt.float32
BF16 = mybir.dt.bfloat16
AF = mybir.ActivationFunctionType
ALU = mybir.AluOpType

NCORES = 8
NG = 16            # grid bins
NQ = 17            # grid corners per axis
IX = 32
IZ = 32
OUT = 64
B = 512
OSH = OUT // NCORES          # outputs per core = 8
K = NQ * IZ                  # 544 contraction rows (q,iz)
M = NG * IX                  # 512 folded free cols (p<=15, ix)
BIG = 1e30
KCH = [(0, 128), (128, 128), (256, 128), (384, 128), (512, 32)]
NBC = B // 128               # 4 batch chunks
ZW = B + 8                   # zv row width (z values + 8 coef columns)
UW = K + 32                  # zrpad row width
NWARM = 28                   # PE warmup matmuls (p-state ramp + DMA cover)

_NC_CACHE = {}


def _build_nc(bd, il):
    """Build the single-core Bass/Tile program (identical across cores)."""
    bd = [float(v) for v in bd]
    il = [float(v) for v in il]

    nc = bacc_mod.Bacc(None, target_bir_lowering=False)
    gmain_d = nc.dram_tensor("gmain", [OSH, 128, 4 * M], BF16, kind="ExternalInput")
    gtail_d = nc.dram_tensor("gtail", [32, OSH * M], BF16, kind="ExternalInput")
    grt_d = nc.dram_tensor("grt", [128, 5 * 32], BF16, kind="ExternalInput")
    zv_d = nc.dram_tensor("zv", [128, 5 * ZW], F32, kind="ExternalInput")
    zrpad_d = nc.dram_tensor("zrpad", [128, 4 * UW], F32, kind="ExternalInput")
    out_d = nc.dram_tensor("out", [B, OSH], F32, kind="ExternalOutput")
    side_d = nc.dram_tensor("side", [OSH, B], F32, kind="ExternalOutput")

    with tile.TileContext(nc) as tc:
        with (
            tc.tile_pool(name="per", bufs=1) as per,       # persistent tiles
            tc.tile_pool(name="tmp", bufs=3) as tmp,       # V/U build scratch
            tc.tile_pool(name="scb", bufs=3) as scb,       # stage2 ACT copies
            tc.tile_pool(name="junk", bufs=2) as junk,     # stt mandatory outs (bf16)
            tc.tile_pool(name="junk32", bufs=2) as junk32, # stt mandatory outs (f32)
            tc.tile_pool(name="ps", bufs=6, space="PSUM") as ps,
            tc.tile_pool(name="ps2", bufs=2, space="PSUM") as ps2,
        ):
            # ---------------- PE warmup ----------------
            # Dependency-free dummy matmuls keep the PE busy through the DMA
            # and V/U build phase so the p-state ramp reaches 2.4 GHz before
            # the real chains start.
            wt = per.tile([128, 128], BF16, tag="warm", name="wt")
            nc.vector.memset(wt[:], 0.0)
            wps = ps2.tile([128, 128], F32, tag="w", name="wps")
            for _ in range(NWARM):
                nc.tensor.matmul(wps[:], wt[:], wt[:], start=True, stop=True)

            # ---------------- input loads (sync queue, priority order) -------
            G_sb = [per.tile([128, 4 * M], BF16, tag=f"G{o}", name=f"G{o}")
                    for o in range(OSH)]
            Gt_sb = per.tile([32, OSH * M], BF16, tag="Gt", name="Gt")
            zvP = per.tile([128, 5 * ZW], F32, tag="zvP", name="zvP")
            zrP = per.tile([128, 4 * UW], F32, tag="zrP", name="zrP")
            grt_sb = per.tile([128, 5 * 32], BF16, tag="grt", name="grt")

            nc.sync.dma_start(G_sb[0][:], gmain_d[0])
            nc.sync.dma_start(zvP[:], zv_d[:, :])
            nc.sync.dma_start(grt_sb[:], grt_d[:, :])
            nc.sync.dma_start(zrP[:], zrpad_d[:, :])
            nc.sync.dma_start(Gt_sb[:], gtail_d[:, :])
            for o in range(1, OSH):
                nc.sync.dma_start(G_sb[o][:], gmain_d[o])

            outT_sb = [per.tile([128, OSH], F32, tag=f"outT{bc}", name=f"outT{bc}")
                       for bc in range(NBC)]

            # ---------------- build V[(q,iz), b] (bf16) ----------------
            # rows r = q*32 + iz;  L = (z - bd[q-1])*il[q-1], R = (bd[q+1] - z)*il[q]
            # V = min(relu(L), relu(R)); per-row (scale,bias) in zv coef cols.
            V_sb = []
            for kci in range(4):
                c0 = kci * ZW
                vt = per.tile([128, B], BF16, tag=f"V{kci}", name=f"V{kci}")
                zin = zvP[:, c0:c0 + B]
                lp = tmp.tile([128, B], BF16, tag="tmp", name=f"vL{kci}")
                nc.scalar.activation(lp[:], zin, AF.Relu,
                                     bias=zvP[:, c0 + B + 1:c0 + B + 2],
                                     scale=zvP[:, c0 + B + 0:c0 + B + 1])
                rp = tmp.tile([128, B], BF16, tag="tmp", name=f"vR{kci}")
                nc.scalar.activation(rp[:], zin, AF.Relu,
                                     bias=zvP[:, c0 + B + 3:c0 + B + 4],
                                     scale=zvP[:, c0 + B + 2:c0 + B + 3])
                nc.vector.tensor_tensor(vt[:], lp[:], rp[:], ALU.min)
                if kci == 0:
                    # q=1 keeps L un-relu'd (left linear extrapolation).
                    # Both min operands must share base partition 0, so the
                    # R arm is recomputed into a [32,B] tile.
                    fl = tmp.tile([32, B], BF16, tag="fix", name="vfq1L")
                    nc.scalar.activation(fl[:], zvP[32:64, c0:c0 + B], AF.Copy,
                                         bias=float(-bd[0] * il[0]), scale=float(il[0]))
                    fr1 = tmp.tile([32, B], BF16, tag="fix", name="vfq1R")
                    nc.scalar.activation(fr1[:], zvP[32:64, c0:c0 + B], AF.Relu,
                                         bias=zvP[32:64, c0 + B + 4:c0 + B + 5],
                                         scale=float(-il[1]))
                    nc.vector.tensor_tensor(vt[32:64, :], fl[:], fr1[:], ALU.min)
                if kci == 3:
                    # q=15 keeps R un-relu'd (right linear extrapolation)
                    fl15 = tmp.tile([32, B], BF16, tag="fix", name="vfq15L")
                    nc.scalar.activation(fl15[:], zvP[96:128, c0:c0 + B], AF.Relu,
                                         bias=zvP[96:128, c0 + B + 4:c0 + B + 5],
                                         scale=float(il[14]))
                    fr = tmp.tile([32, B], BF16, tag="fix", name="vfq15R")
                    nc.scalar.activation(fr[:], zvP[96:128, c0:c0 + B], AF.Copy,
                                         bias=float(bd[16] * il[15]), scale=float(-il[15]))
                    nc.vector.tensor_tensor(vt[96:128, :], fl15[:], fr[:], ALU.min)
                V_sb.append(vt)
            # q=16 rows: hat_16 = relu((z - bd[15])*il[15]); no right arm.
            c0 = 4 * ZW
            vt4 = per.tile([32, B], BF16, tag="V4", name="V4")
            nc.scalar.activation(vt4[:], zvP[0:32, c0:c0 + B], AF.Relu,
                                 bias=zvP[0:32, c0 + B + 1:c0 + B + 2],
                                 scale=zvP[0:32, c0 + B + 0:c0 + B + 1])
            V_sb.append(vt4)

            # ---------------- side term: side[o,b] = sum_k gr[k,o] V[k,b] ----
            # grt columns padded 8 -> 32; psum [32, 512], rows 8..31 junk.
            psS = ps2.tile([32, B], F32, tag="w", name="psS")
            for kci, (r0, rows) in enumerate(KCH):
                nc.tensor.matmul(psS[:], grt_sb[0:rows, kci * 32:(kci + 1) * 32],
                                 V_sb[kci][0:rows, 0:B],
                                 start=(kci == 0), stop=(kci == len(KCH) - 1))
            sideS = per.tile([OSH, B], F32, tag="sideS", name="sideS")
            nc.scalar.activation(sideS[:], psS[0:OSH, :], AF.Copy)
            nc.sync.dma_start(side_d[:, :], sideS[:])

            # ---------------- build U[b, (p<=15,ix)] (f32 + bf16 copy) -------
            # zrpad col 32+p*32+ix = R_p(x[ix,b]); identity L_p = 1 - R_{p-1}
            # gives the L operand as a 32-col-shifted read of the same tensor.
            U32_sb, U16_sb = [], []
            for bc in range(NBC):
                c0 = bc * UW
                u32 = per.tile([128, M], F32, tag=f"U32_{bc}", name=f"U32_{bc}")
                u16 = per.tile([128, M], BF16, tag=f"U16_{bc}", name=f"U16_{bc}")
                lp = tmp.tile([128, M], F32, tag="tmpu", name=f"uL{bc}")
                nc.scalar.activation(lp[:], zrP[:, c0:c0 + M], AF.Relu,
                                     bias=1.0, scale=-1.0)
                rp = tmp.tile([128, M], F32, tag="tmpu", name=f"uR{bc}")
                nc.scalar.activation(rp[:], zrP[:, c0 + 32:c0 + M + 32], AF.Relu)
                nc.vector.tensor_tensor(u32[:], lp[:], rp[:], ALU.min)
                # p=1 fix: L un-relu'd
                fx = tmp.tile([128, 32], F32, tag="tmpu", name=f"ufix{bc}")
                nc.scalar.activation(fx[:], zrP[:, c0 + 32:c0 + 64], AF.Copy,
                                     bias=1.0, scale=-1.0)
                nc.vector.tensor_tensor(u32[:, 32:64], fx[:], rp[:, 32:64], ALU.min)
                # p=15 fix: R un-relu'd (raw f32 read)
                nc.vector.tensor_tensor(u32[:, 480:512], lp[:, 480:512],
                                        zrP[:, c0 + 512:c0 + 544], ALU.min)
                nc.vector.tensor_copy(u16[:], u32[:])
                U32_sb.append(u32)
                U16_sb.append(u16)

            # ---------------- main loop: 5-matmul chains + fused reduce ------
            for o in range(OSH):
                for bc in range(NBC):
                    bs = slice(bc * 128, (bc + 1) * 128)
                    pst = ps.tile([128, M], F32, tag="ps", name=f"ps{o}_{bc}")
                    for kci in range(4):
                        nc.tensor.matmul(pst[:], V_sb[kci][:, bs],
                                         G_sb[o][:, kci * M:(kci + 1) * M],
                                         start=(kci == 0), stop=False)
                    nc.tensor.matmul(pst[:], V_sb[4][0:32, bs],
                                     Gt_sb[0:32, o * M:(o + 1) * M],
                                     start=False, stop=True)

                    if (o * NBC + bc) % 2 == 0:
                        # path A: ACT copies PSUM -> SBUF bf16, DVE 2x stt
                        cp = scb.tile([128, M], BF16, tag="scb", name=f"cp{o}_{bc}")
                        nc.scalar.activation(cp[:], pst[:], AF.Copy)
                        jk = junk.tile([128, M], BF16, tag="junk", name=f"jk{o}_{bc}")
                        nc.vector.scalar_tensor_tensor(
                            out=jk[:], in0=cp[:], scalar=1.0, in1=U16_sb[bc][:],
                            op0=ALU.mult, op1=ALU.mult,
                            accum_out=outT_sb[bc][:, o:o + 1])
                    else:
                        # path B: DVE reduces directly out of PSUM (fp32)
                        jk = junk32.tile([128, M], F32, tag="junk32", name=f"jk{o}_{bc}")
                        nc.vector.scalar_tensor_tensor(
                            out=jk[:], in0=pst[:], scalar=1.0, in1=U32_sb[bc][:],
                            op0=ALU.mult, op1=ALU.mult,
                            accum_out=outT_sb[bc][:, o:o + 1])

            # ---------------- store ----------------
            for bc in range(NBC):
                nc.sync.dma_start(out_d[bc * 128:(bc + 1) * 128, :], outT_sb[bc][:])

    nc.finalize()
    return nc


def _host_prep(x, z, func_parameter, borders, il):
    import ml_dtypes
    x = np.asarray(x, np.float32)
    z = np.asarray(z, np.float32)
    F = np.asarray(func_parameter, np.float32)
    bd = np.asarray(borders, np.float32)
    il = np.asarray(il, np.float32)
    bf = ml_dtypes.bfloat16

    # G_all[o, q*32+iz, p*32+ix] = F[p,q,o,ix,iz]
    G_all = np.ascontiguousarray(F.transpose(2, 1, 4, 0, 3)).reshape(OUT, K, K)
    # x-side partition-of-unity fold: drop p=16 columns
    Ghat = G_all[:, :, 0:M] - np.tile(G_all[:, :, M:K], (1, 1, NG))
    gr = G_all[:, :, M:K].sum(axis=2)                         # [64, 544]

    # zv rows: [z replicated | aL bL aR bR 0 0 0 0], packed [128, 5*ZW]
    zrep = np.tile(z, (NQ, 1))                                # [544, 512]
    q = np.arange(NQ)
    aL = np.where(q >= 1, il[np.clip(q - 1, 0, NG - 1)], 0.0)
    bL = np.where(q >= 1, -bd[np.clip(q - 1, 0, NQ - 1)] * il[np.clip(q - 1, 0, NG - 1)], BIG)
    aR = np.where(q <= NG - 1, -il[np.clip(q, 0, NG - 1)], 0.0)
    bR = np.where(q <= NG - 1, bd[np.clip(q + 1, 0, NQ - 1)] * il[np.clip(q, 0, NG - 1)], BIG)
    fixb = np.zeros(NQ)
    fixb[1] = bd[2] * il[1]          # q=1 fix: R-side relu bias
    fixb[15] = -bd[14] * il[14]      # q=15 fix: L-side relu bias
    zero = np.zeros(NQ)
    vcoef = np.ascontiguousarray(
        np.stack([np.repeat(c.astype(np.float32), IZ)
                  for c in (aL, bL, aR, bR, fixb, zero, zero, zero)], axis=1))
    zv = np.concatenate([zrep, vcoef], axis=1)                # [544, 520]
    zvP = np.zeros((128, 5 * ZW), np.float32)
    for kci, (r0, rows) in enumerate(KCH):
        zvP[0:rows, kci * ZW:(kci + 1) * ZW] = zv[r0:r0 + rows]

    # zrpad rows: [-BIG pad x32 | R_p(x[ix,b]) cols], packed [128, 4*UW]
    Rx = np.empty((NQ, IX, B), np.float32)
    for p in range(NG):
        Rx[p] = (bd[p + 1] - x) * il[p]
    Rx[NG] = BIG
    ZR_T = Rx.transpose(2, 0, 1).reshape(B, K)
    zrpad = np.concatenate([np.full((B, 32), -BIG, np.float32), ZR_T], axis=1)
    zrP = np.zeros((128, 4 * UW), np.float32)
    for bc in range(NBC):
        zrP[:, bc * UW:(bc + 1) * UW] = zrpad[bc * 128:(bc + 1) * 128]

    gmain_all, gtail_all, grt_all = [], [], []
    for c in range(NCORES):
        Go = Ghat[c * OSH:(c + 1) * OSH]                      # [8, 544, 512]
        gmain = np.ascontiguousarray(
            Go[:, 0:512, :].reshape(OSH, 4, 128, M).transpose(0, 2, 1, 3)
            .reshape(OSH, 128, 4 * M)).astype(bf)
        gtail = np.ascontiguousarray(
            Go[:, 512:K, :].transpose(1, 0, 2).reshape(32, OSH * M)).astype(bf)
        grc = gr[c * OSH:(c + 1) * OSH]                       # [8, 544]
        grtP = np.zeros((128, 5 * 32), np.float32)
        for kci, (r0, rows) in enumerate(KCH):
            grtP[0:rows, kci * 32:kci * 32 + OSH] = grc[:, r0:r0 + rows].T
        gmain_all.append(gmain)
        gtail_all.append(gtail)
        grt_all.append(grtP.astype(bf))
    return gmain_all, gtail_all, grt_all, zvP, zrP, bd, il


def kernel(x, z, func_parameter, borders, inverse_chunk_lengths, _trace=False):
    gmain_all, gtail_all, grt_all, zvP, zrP, bd, il = _host_prep(
        x, z, func_parameter, borders, inverse_chunk_lengths)

    key = (bd.tobytes(), il.tobytes())
    if key not in _NC_CACHE:
        _NC_CACHE[key] = _build_nc(bd, il)
    nc = _NC_CACHE[key]

    in_maps = []
    for c in range(NCORES):
        in_maps.append({
            "gmain": gmain_all[c],
            "gtail": gtail_all[c],
            "grt": grt_all[c],
            "zv": zvP,
            "zrpad": zrP,
        })

    res = run_bass_kernel_spmd(nc, in_maps, core_ids=list(range(NCORES)),
                               trace=_trace)
    parts = []
    for c in range(NCORES):
        r = res.results[c]
        parts.append(r["out"].T.astype(np.float32) + r["side"].astype(np.float32))
    out = np.ascontiguousarray(np.concatenate(parts, axis=0).astype(np.float32))
    if _trace:
        return out, res
    return out
